# revision 15
# baseline (speedup 1.0000x reference)
"""Multi-head attention (B=2, S=2048, D=1024, H=16, dh=64) on 8 TRN2 NeuronCores.

Sharding: data-parallel over batch (2) x tensor-parallel over heads (4 per core).
Core c handles batch c//4 and heads [4*(c%4), 4*(c%4)+4). Each core computes a
partial output (its heads' contribution through Wo); the host sums the 4 partials
per batch and adds bo.

Pipeline (v3): the softmax exp on the scalar (ACT) engine is the throughput
floor (131072 elems/lane @ 1.2 GHz ~= 110us); everything is arranged so ACT
starts early and never starves:
 - DMAs ordered by need (wk, xk0, wq, xq0, ...); PE warmed on dummy matmuls
   against Wk while the first kT chunk lands (HAM clock-gate releases after
   ~3.4us of activity), K-projection interleaved with chunk 0's QK groups so
   the first exp fires ~25us in.
 - QK head pairs at PE row-tiles 0/64 (contraction=64) via tile_position
   auto-derive, emitted interleaved -> ~2x concurrency.
 - Next chunk's QK pairs are emitted BEFORE this chunk's PV so the in-order
   PE queue keeps feeding ACT across chunk boundaries; PV/normalize/out-proj
   of chunk c execute during chunk c+1's exp stream.
 - softmax denominators via the ones-column in the PV stationary; fast
   custom-DVE reciprocal (5x the iterative divide) from an SBUF staging copy
   (custom-DVE ops misread PSUM sources - hardware-verified).
 - Output projection + DMA streamed per chunk; final chunk's PSUM
   evacuations split between ACT (idle by then) and DVE to shorten the tail.

Matmuls in bf16 with f32 PSUM accumulation.
"""

import sys

if "/opt/trn_rl_repo" not in sys.path:
    sys.path.insert(0, "/opt/trn_rl_repo")

import ml_dtypes
import numpy as np

import concourse.bass as bass
import concourse.mybir as mybir
import concourse.tile as tile
from concourse import bacc, bass_utils
from concourse.bass import ts

# Problem constants (hardcoded per contract)
B, S, D = 2, 2048, 1024
H, DH = 16, 64            # total heads, head dim
HC = 4                    # heads per core
DHC = HC * DH             # 256 projected dims per core
NCORES = 8
P = 128
CH = 512                  # query-chunk for attention / projection sub-chunk
NCH = S // CH             # 4 (q-projection chunks)
# attention chunks: last 512 queries split in two so the end-of-kernel
# dependency chain (PV of the final head pair -> normalize -> out-proj ->
# evac/DMA) covers half the width
AC = [(0, 512), (512, 512), (1024, 512), (1536, 256), (1792, 256)]
NAC = len(AC)
TT = S // P               # 16 key tiles
KO = D // P               # 8 contraction tiles for projections

f32 = mybir.dt.float32
bf16 = mybir.dt.bfloat16
EXP = mybir.ActivationFunctionType.Exp
i16 = mybir.dt.int16
MULT = mybir.AluOpType.mult
ADD = mybir.AluOpType.add
# Schraudolph fast-exp in bf16 bit space: bf16_bits(e^x) ~= round(x*128/ln2
# + 127*128 - C). 3 of 8 key-tile groups per head are computed this way on
# the (otherwise slack) DVE, cutting the ACT exp stream by 3/8. End-to-end
# rel err simulated at 1.29e-2 vs the 2e-2 gate. EARLY tb groups are the
# offloaded ones: a chunk's final tiles must be consumed promptly (they gate
# the l_ps ring for the next chunk's QK) and ACT consumes on time while the
# in-order DVE queue lags.
SEXP_OFF = (0, 2, 4)
SEXP_A = float(128.0 / np.log(2.0))
SEXP_B = float(127.0 * 128.0 - 5.75)

_compiled = None          # cached nc across calls
last_results = None       # BassKernelResults of the most recent run (for profiling)


def _build():
    nc = bacc.Bacc("TRN2", target_bir_lowering=False, debug=False)

    # Per-core DRAM parameters. Activations are pre-transposed on host.
    qT = nc.dram_tensor("qT", [D, S], bf16, kind="ExternalInput")
    kT = nc.dram_tensor("kT", [D, S], bf16, kind="ExternalInput")
    vT = nc.dram_tensor("vT", [D, S], bf16, kind="ExternalInput")
    wq = nc.dram_tensor("wq", [D, DHC], bf16, kind="ExternalInput")
    wk = nc.dram_tensor("wk", [D, DHC], bf16, kind="ExternalInput")
    wv = nc.dram_tensor("wv", [D, DHC], bf16, kind="ExternalInput")
    wo = nc.dram_tensor("wo", [DHC, D], bf16, kind="ExternalInput")
    bq = nc.dram_tensor("bq", [DHC], f32, kind="ExternalInput")
    bk = nc.dram_tensor("bk", [DHC], f32, kind="ExternalInput")
    bv = nc.dram_tensor("bv", [DHC], f32, kind="ExternalInput")
    out = nc.dram_tensor("out", [S, D], f32, kind="ExternalOutput")

    qT_r = qT.ap().rearrange("(ko p) s -> p ko s", p=P)
    kT_r = kT.ap().rearrange("(ko p) s -> p ko s", p=P)
    vT_r = vT.ap().rearrange("(ko p) s -> p ko s", p=P)

    with tile.TileContext(nc) as tc:
        with (
            tc.tile_pool(name="weights", bufs=1) as wpool,
            tc.tile_pool(name="acts", bufs=1) as apool,
            tc.tile_pool(name="xin", bufs=2) as xpool,
            tc.tile_pool(name="pt", bufs=7) as ptpool,
            tc.tile_pool(name="small", bufs=2) as spool,
            tc.tile_pool(name="outs", bufs=4) as opool,
            tc.tile_pool(name="shared_ps", bufs=2, space="PSUM") as shared_ps,
            tc.tile_pool(name="l_ps", bufs=3, space="PSUM") as l_ps,
        ):
            # Preload the exp table set while DMAs run (first ACTIVATE to a
            # new set pays ~2.7us of ACT_TABLE_LOAD).
            warm = wpool.tile([1, 8], f32, tag="warm")
            nc.vector.memset(warm, 0.0)
            warm2 = wpool.tile([1, 8], f32, tag="warm2")
            nc.scalar.activation(out=warm2, in_=warm, func=EXP)

            # ---- weights, in consumption order ----
            wq_sb = wpool.tile([P, KO, DHC], bf16, tag="wq")
            wk_sb = wpool.tile([P, KO, DHC], bf16, tag="wk")
            wv_sb = wpool.tile([P, KO, DHC], bf16, tag="wv")
            wo_sb = wpool.tile([P, DHC // P, D], bf16, tag="wo")
            bq_sb = wpool.tile([P, 2], f32, tag="bq")
            bk_sb = wpool.tile([P, 2], f32, tag="bk")
            bv_row = wpool.tile([P, DHC], f32, tag="bv_row")
            bv_bc = wpool.tile([P, DHC], f32, tag="bv_bc")
            bv_heads = bv_bc[:, :].rearrange("p (h c) -> p h c", c=DH)

            nc.sync.dma_start(out=wk_sb, in_=wk.ap().rearrange("(ko p) m -> p ko m", p=P))
            nc.sync.dma_start(out=bk_sb, in_=bk.ap().rearrange("(mo p) -> p mo", p=P))

            # PE warm-up: dummy matmuls on Wk as soon as it lands, so the HAM
            # clock gate is released (2.4 GHz) by the time real work starts.
            wups = shared_ps.tile([P, DHC], f32, tag="ps")
            for i in range(16):
                nc.tensor.matmul(wups, wk_sb[:, 0, 0:P], wk_sb[:, i % KO, :],
                                 start=True, stop=True, skip_group_check=True)

            # ---- activation tiles ----
            q_sb = apool.tile([P, 2, S], bf16, tag="q")
            k_sb = apool.tile([P, 2, S], bf16, tag="k")
            VW = HC * (DH + 1)
            v_sb = apool.tile([P, TT, VW + P - (DH + 1)], bf16, tag="v")
            attn_sb = apool.tile([P, 2, S], bf16, tag="attn")
            v_heads = v_sb[:, :, 0:VW].rearrange("p tt (h c) -> p tt h c", c=DH + 1)

            # ---- emission helpers ----
            def kproj(c):
                sl = slice(c * CH, (c + 1) * CH)
                xk = xpool.tile([P, KO, CH], bf16, tag="x", name=f"xk{c}")
                nc.sync.dma_start(out=xk, in_=kT_r[:, :, sl])
                for m in range(2):
                    ps = shared_ps.tile([P, CH], f32, tag="ps")
                    for ko in range(KO):
                        nc.tensor.matmul(ps, wk_sb[:, ko, ts(m, P)], xk[:, ko, :],
                                         start=(ko == 0), stop=(ko == KO - 1))
                    nc.vector.tensor_scalar_add(out=k_sb[:, m, sl], in0=ps,
                                                scalar1=bk_sb[:, m : m + 1])

            def qproj(c):
                sl = slice(c * CH, (c + 1) * CH)
                xq = xpool.tile([P, KO, CH], bf16, tag="x", name=f"xq{c}")
                nc.sync.dma_start(out=xq, in_=qT_r[:, :, sl])
                for m in range(2):
                    ps = shared_ps.tile([P, CH], f32, tag="ps")
                    for ko in range(KO):
                        nc.tensor.matmul(ps, wq_sb[:, ko, ts(m, P)], xq[:, ko, :],
                                         start=(ko == 0), stop=(ko == KO - 1))
                    nc.vector.tensor_scalar_add(out=q_sb[:, m, sl], in0=ps,
                                                scalar1=bq_sb[:, m : m + 1])

            def vproj(c):
                sl = slice(c * CH, (c + 1) * CH)
                xv = xpool.tile([P, KO, CH], bf16, tag="x", name=f"xv{c}")
                nc.sync.dma_start(out=xv, in_=vT_r[:, :, sl])
                for th in range(CH // P):
                    tt = (c * CH) // P + th
                    ps = shared_ps.tile([P, DHC], f32, tag="ps")
                    for ko in range(KO):
                        nc.tensor.matmul(ps, xv[:, ko, ts(th, P)], wv_sb[:, ko, :],
                                         start=(ko == 0), stop=(ko == KO - 1))
                    nc.vector.tensor_add(
                        out=v_heads[:, tt, :, 0:DH],
                        in0=ps.rearrange("p (h c) -> p h c", c=DH),
                        in1=bv_heads,
                    )

            # pts[c][h] SBUF tiles of exp'd logits (keys on partitions)
            pts = [dict() for _ in range(NAC)]

            def alloc_pts(c, heads):
                for h in heads:
                    pts[c][h] = ptpool.tile([P, TT, AC[c][1]], bf16, tag="pt",
                                            name=f"pt_c{c}_h{h}")

            def qk_tbs(c, m, tbs):
                """QK + exp for heads (2m, 2m+1) of attention chunk c."""
                qlo, qw = AC[c]
                csl = slice(qlo, qlo + qw)
                h0, h1 = 2 * m, 2 * m + 1
                for tb in tbs:
                    psA = l_ps.tile([P, 2, qw], f32, tag="l")
                    psB = l_ps.tile([P, 2, qw], f32, tag="l")
                    for j in range(2):
                        tt = 2 * tb + j
                        nc.tensor.matmul(
                            psA[:, j, :],
                            k_sb[0:DH, m, ts(tt, P)],
                            q_sb[0:DH, m, csl],
                            start=True, stop=True,
                        )
                        nc.tensor.matmul(
                            psB[:, j, :],
                            k_sb[DH : 2 * DH, m, ts(tt, P)],
                            q_sb[DH : 2 * DH, m, csl],
                            start=True, stop=True,
                        )
                    for ps_t, hh in ((psA, h0), (psB, h1)):
                        dst = pts[c][hh][:, 2 * tb : 2 * tb + 2, :]
                        if tb in SEXP_OFF:
                            nc.vector.tensor_scalar(
                                out=dst.bitcast(i16), in0=ps_t,
                                scalar1=SEXP_A, scalar2=SEXP_B,
                                op0=MULT, op1=ADD)
                        else:
                            nc.scalar.activation(out=dst, in_=ps_t, func=EXP)

            def pv_head(c, h):
                """PV (+ denominator via ones column) and normalize for head h."""
                qlo, qw = AC[c]
                csl = slice(qlo, qlo + qw)
                base = DH * (h % 2)
                m = h // 2
                po = shared_ps.tile([P, qw], f32, tag="ps")
                for tt in range(TT):
                    nc.tensor.matmul(
                        po[0 : DH + 1, :],
                        v_heads[:, tt, h, :],
                        pts[c][h][:, tt, :],
                        start=(tt == 0), stop=(tt == TT - 1),
                    )
                # stage the denominator row at partition 0: the custom-DVE
                # reciprocal misreads inputs at base_partition != 0 (and PSUM
                # sources) - both hardware-verified
                den = spool.tile([1, qw], f32, tag="den")
                nc.vector.tensor_copy(out=den[0:1, :], in_=po[DH : DH + 1, :])
                rec = spool.tile([1, qw], f32, tag="rec")
                nc.vector.reciprocal_approx_fast(out=rec[0:1, :], in_=den[0:1, :])
                bc = spool.tile([P, qw], f32, tag="bc")
                nc.gpsimd.partition_broadcast(bc[0:DH, :], rec[0:1, :])
                nc.vector.tensor_mul(
                    out=attn_sb[base : base + DH, m, csl],
                    in0=po[0:DH, :], in1=bc[0:DH, :],
                )

            def outproj(c, last=False):
                tail_evac = c >= NAC - 2
                # Out-proj runs entirely through the l_ps (QK logits) ring:
                # its WAR dependencies defer these matmuls to the next chunk
                # boundary, where the evacuations no longer gate anything in
                # the ACT/DVE queues (shared_ps evacs stalled the exp stream
                # ~15us at every boundary). Evacs are DVE-only mid-stream so
                # ACT stays a pure exp queue; the idle ACT helps in the tail.
                qlo, qw = AC[c]
                for sti in range(qw // P):
                    st = qlo // P + sti
                    pw2 = l_ps.tile([P, 2, CH], f32, tag="l")
                    for n in range(2):
                        for ko in range(2):
                            nc.tensor.matmul(pw2[:, n, :],
                                             attn_sb[:, ko, ts(st, P)],
                                             wo_sb[:, ko, ts(n, 512)],
                                             start=(ko == 0), stop=(ko == 1))
                    for n in range(2):
                        ot = opool.tile([P, 512], f32, tag="ot")
                        if tail_evac and n == 1:
                            # chunks 2/3 evacuate in the tail where ACT is idle
                            nc.scalar.copy(out=ot, in_=pw2[:, n, :])
                        else:
                            nc.vector.tensor_copy(out=ot, in_=pw2[:, n, :])
                        # sync queue is idle once inputs land; gpsimd triggers
                        # (644ns each) were delaying the normalize broadcasts
                        nc.sync.dma_start(out=out.ap()[ts(st, P), ts(n, 512)], in_=ot)

            # ---- emission schedule ----
            # Startup: K-projection chunks interleaved with chunk 0's QK
            # groups so exp starts as soon as the first key chunk is ready.
            kproj(0)
            nc.sync.dma_start(out=wq_sb, in_=wq.ap().rearrange("(ko p) m -> p ko m", p=P))
            nc.sync.dma_start(out=bq_sb, in_=bq.ap().rearrange("(mo p) -> p mo", p=P))
            qproj(0)
            alloc_pts(0, range(HC))
            qk_tbs(0, 0, [0, 1]); qk_tbs(0, 1, [0, 1])
            kproj(1)
            qk_tbs(0, 0, [2, 3]); qk_tbs(0, 1, [2, 3])
            kproj(2)
            qk_tbs(0, 0, [4, 5]); qk_tbs(0, 1, [4, 5])
            kproj(3)
            qk_tbs(0, 0, [6, 7]); qk_tbs(0, 1, [6, 7])

            ones_f32 = wpool.tile([P, TT, HC], f32, tag="ones")
            nc.vector.memset(ones_f32, 1.0)
            nc.vector.tensor_copy(out=v_heads[:, :, :, DH], in_=ones_f32)
            nc.vector.memset(v_sb[:, :, VW:], 0.0)

            # Steady state: QK of chunk c+1 ahead of PV of chunk c in the
            # in-order PE queue; PV/normalize/out of chunk c execute while
            # ACT streams chunk c+1's exps. V projection rides inside
            # iteration 0, after xq1's DMA is already queued.
            qproj_done = {0}
            for c in range(NAC):
                if c + 1 < NAC:
                    qi = AC[c + 1][0] // CH
                    if qi not in qproj_done:
                        qproj(qi)
                        qproj_done.add(qi)
                    alloc_pts(c + 1, range(HC))
                    qk_tbs(c + 1, 0, range(TT // 2))
                if c == 0:
                    nc.sync.dma_start(out=wv_sb, in_=wv.ap().rearrange("(ko p) m -> p ko m", p=P))
                    nc.sync.dma_start(out=bv_row[0:1, :], in_=bv.ap().rearrange("(a d) -> a d", a=1))
                    nc.gpsimd.partition_broadcast(bv_bc, bv_row[0:1, :])
                    for vc in range(NCH):
                        vproj(vc)
                    nc.sync.dma_start(out=wo_sb, in_=wo.ap().rearrange("(ko p) n -> p ko n", p=P))
                if not (c == NAC - 1):
                    pv_head(c, 0)
                    pv_head(c, 1)
                if c + 1 < NAC:
                    qk_tbs(c + 1, 1, range(TT // 2))
                pv_head(c, 2)
                pv_head(c, 3)
                if c + 1 == NAC - 1:
                    # final chunk's first head-pair PV runs during its exp
                    # stream instead of serializing into the tail
                    pv_head(c + 1, 0)
                    pv_head(c + 1, 1)
                outproj(c, last=(c == NAC - 1))

    nc.finalize()
    return nc


def kernel(**inputs):
    global _compiled, last_results
    if _compiled is None:
        _compiled = _build()
    nc = _compiled

    query = np.asarray(inputs["query"], np.float32)
    key = np.asarray(inputs["key"], np.float32)
    value = np.asarray(inputs["value"], np.float32)
    Wq = np.asarray(inputs["Wq"], np.float32)
    Wk = np.asarray(inputs["Wk"], np.float32)
    Wv = np.asarray(inputs["Wv"], np.float32)
    Wo = np.asarray(inputs["Wo"], np.float32)
    bq_f = np.asarray(inputs["bq"], np.float32)
    bk_f = np.asarray(inputs["bk"], np.float32)
    bv_f = np.asarray(inputs["bv"], np.float32)
    bo_f = np.asarray(inputs["bo"], np.float32)

    bf = ml_dtypes.bfloat16
    scale = 1.0 / np.sqrt(np.float32(DH))
    qT = [np.ascontiguousarray(query[b].T).astype(bf) for b in range(B)]
    kT = [np.ascontiguousarray(key[b].T).astype(bf) for b in range(B)]
    vT = [np.ascontiguousarray(value[b].T).astype(bf) for b in range(B)]

    in_maps = []
    for c in range(NCORES):
        b = c // 4
        sh = c % 4
        sl = slice(DHC * sh, DHC * (sh + 1))
        in_maps.append({
            "qT": qT[b], "kT": kT[b], "vT": vT[b],
            "wq": (Wq[:, sl] * scale).astype(bf),
            "wk": np.ascontiguousarray(Wk[:, sl]).astype(bf),
            "wv": np.ascontiguousarray(Wv[:, sl]).astype(bf),
            "wo": np.ascontiguousarray(Wo[sl, :]).astype(bf),
            "bq": np.ascontiguousarray(bq_f[sl]) * scale,
            "bk": np.ascontiguousarray(bk_f[sl]),
            "bv": np.ascontiguousarray(bv_f[sl]),
        })

    res = bass_utils.run_bass_kernel_spmd(nc, in_maps, core_ids=list(range(NCORES)))
    last_results = res

    final = np.empty((B, S, D), np.float32)
    for b in range(B):
        acc = res.results[4 * b]["out"].astype(np.float32)
        for sh in range(1, 4):
            acc = acc + res.results[4 * b + sh]["out"]
        final[b] = acc + bo_f
    return final


# revision 17
# speedup vs baseline: 1.0385x; 1.0385x over previous
"""Multi-head attention (B=2, S=2048, D=1024, H=16, dh=64) on 8 TRN2 NeuronCores.

Sharding: data-parallel over batch (2) x tensor-parallel over heads (4 per core).
Core c handles batch c//4 and heads [4*(c%4), 4*(c%4)+4). Each core computes a
partial output (its heads' contribution through Wo); the host sums the 4 partials
per batch and adds bo.

Pipeline (v3): the softmax exp on the scalar (ACT) engine is the throughput
floor (131072 elems/lane @ 1.2 GHz ~= 110us); everything is arranged so ACT
starts early and never starves:
 - DMAs ordered by need (wk, xk0, wq, xq0, ...); PE warmed on dummy matmuls
   against Wk while the first kT chunk lands (HAM clock-gate releases after
   ~3.4us of activity), K-projection interleaved with chunk 0's QK groups so
   the first exp fires ~25us in.
 - QK head pairs at PE row-tiles 0/64 (contraction=64) via tile_position
   auto-derive, emitted interleaved -> ~2x concurrency.
 - Next chunk's QK pairs are emitted BEFORE this chunk's PV so the in-order
   PE queue keeps feeding ACT across chunk boundaries; PV/normalize/out-proj
   of chunk c execute during chunk c+1's exp stream.
 - softmax denominators via the ones-column in the PV stationary; fast
   custom-DVE reciprocal (5x the iterative divide) from an SBUF staging copy
   (custom-DVE ops misread PSUM sources - hardware-verified).
 - Output projection + DMA streamed per chunk; final chunk's PSUM
   evacuations split between ACT (idle by then) and DVE to shorten the tail.

Matmuls in bf16 with f32 PSUM accumulation.
"""

import sys

if "/opt/trn_rl_repo" not in sys.path:
    sys.path.insert(0, "/opt/trn_rl_repo")

import ml_dtypes
import numpy as np

import concourse.bass as bass
import concourse.mybir as mybir
import concourse.tile as tile
from concourse import bacc, bass_utils
from concourse.bass import ts

# Problem constants (hardcoded per contract)
B, S, D = 2, 2048, 1024
H, DH = 16, 64            # total heads, head dim
HC = 4                    # heads per core
DHC = HC * DH             # 256 projected dims per core
NCORES = 8
P = 128
CH = 512                  # query-chunk for attention / projection sub-chunk
NCH = S // CH             # 4
TT = S // P               # 16 key tiles
KO = D // P               # 8 contraction tiles for projections

f32 = mybir.dt.float32
bf16 = mybir.dt.bfloat16
EXP = mybir.ActivationFunctionType.Exp
i16 = mybir.dt.int16
MULT = mybir.AluOpType.mult
ADD = mybir.AluOpType.add
# Schraudolph fast-exp in bf16 bit space: bf16_bits(e^x) ~= round(x*128/ln2
# + 127*128 - C). 3 of 8 key-tile groups per head are computed this way on
# the (otherwise slack) DVE, cutting the ACT exp stream by 3/8. End-to-end
# rel err simulated at 1.29e-2 vs the 2e-2 gate. EARLY tb groups are the
# offloaded ones: a chunk's final tiles must be consumed promptly (they gate
# the l_ps ring for the next chunk's QK) and ACT consumes on time while the
# in-order DVE queue lags.
SEXP_OFF = (0, 2, 4)
SEXP_A = float(128.0 / np.log(2.0))
SEXP_B = float(127.0 * 128.0 - 5.75)

_compiled = None          # cached nc across calls
last_results = None       # BassKernelResults of the most recent run (for profiling)


def _build():
    nc = bacc.Bacc("TRN2", target_bir_lowering=False, debug=False)

    # Per-core DRAM parameters. Activations are pre-transposed on host.
    qT = nc.dram_tensor("qT", [D, S], bf16, kind="ExternalInput")
    kT = nc.dram_tensor("kT", [D, S], bf16, kind="ExternalInput")
    vT = nc.dram_tensor("vT", [D, S], bf16, kind="ExternalInput")
    wq = nc.dram_tensor("wq", [D, DHC], bf16, kind="ExternalInput")
    wk = nc.dram_tensor("wk", [D, DHC], bf16, kind="ExternalInput")
    wv = nc.dram_tensor("wv", [D, DHC], bf16, kind="ExternalInput")
    wo = nc.dram_tensor("wo", [DHC, D], bf16, kind="ExternalInput")
    bq = nc.dram_tensor("bq", [DHC], f32, kind="ExternalInput")
    bk = nc.dram_tensor("bk", [DHC], f32, kind="ExternalInput")
    bv = nc.dram_tensor("bv", [DHC], f32, kind="ExternalInput")
    out = nc.dram_tensor("out", [S, D], f32, kind="ExternalOutput")

    qT_r = qT.ap().rearrange("(ko p) s -> p ko s", p=P)
    kT_r = kT.ap().rearrange("(ko p) s -> p ko s", p=P)
    vT_r = vT.ap().rearrange("(ko p) s -> p ko s", p=P)

    with tile.TileContext(nc) as tc:
        with (
            tc.tile_pool(name="weights", bufs=1) as wpool,
            tc.tile_pool(name="acts", bufs=1) as apool,
            tc.tile_pool(name="xin", bufs=2) as xpool,
            tc.tile_pool(name="pt", bufs=7) as ptpool,
            tc.tile_pool(name="small", bufs=2) as spool,
            tc.tile_pool(name="outs", bufs=4) as opool,
            tc.tile_pool(name="shared_ps", bufs=2, space="PSUM") as shared_ps,
            tc.tile_pool(name="l_ps", bufs=3, space="PSUM") as l_ps,
        ):
            # Preload the exp table set while DMAs run (first ACTIVATE to a
            # new set pays ~2.7us of ACT_TABLE_LOAD).
            warm = wpool.tile([1, 8], f32, tag="warm")
            nc.vector.memset(warm, 0.0)
            warm2 = wpool.tile([1, 8], f32, tag="warm2")
            nc.scalar.activation(out=warm2, in_=warm, func=EXP)

            # ---- weights, in consumption order ----
            wq_sb = wpool.tile([P, KO, DHC], bf16, tag="wq")
            wk_sb = wpool.tile([P, KO, DHC], bf16, tag="wk")
            wv_sb = wpool.tile([P, KO, DHC], bf16, tag="wv")
            wo_sb = wpool.tile([P, DHC // P, D], bf16, tag="wo")
            bq_sb = wpool.tile([P, 2], f32, tag="bq")
            bk_sb = wpool.tile([P, 2], f32, tag="bk")
            bv_row = wpool.tile([P, DHC], f32, tag="bv_row")
            bv_bc = wpool.tile([P, DHC], f32, tag="bv_bc")
            bv_heads = bv_bc[:, :].rearrange("p (h c) -> p h c", c=DH)

            nc.sync.dma_start(out=wk_sb, in_=wk.ap().rearrange("(ko p) m -> p ko m", p=P))
            nc.sync.dma_start(out=bk_sb, in_=bk.ap().rearrange("(mo p) -> p mo", p=P))

            # PE warm-up: dummy matmuls on Wk as soon as it lands, so the HAM
            # clock gate is released (2.4 GHz) by the time real work starts.
            wups = shared_ps.tile([P, DHC], f32, tag="ps")
            for i in range(16):
                nc.tensor.matmul(wups, wk_sb[:, 0, 0:P], wk_sb[:, i % KO, :],
                                 start=True, stop=True, skip_group_check=True)

            # ---- activation tiles ----
            q_sb = apool.tile([P, 2, S], bf16, tag="q")
            k_sb = apool.tile([P, 2, S], bf16, tag="k")
            VW = HC * (DH + 1)
            v_sb = apool.tile([P, TT, VW + P - (DH + 1)], bf16, tag="v")
            attn_sb = apool.tile([P, 2, S], bf16, tag="attn")
            v_heads = v_sb[:, :, 0:VW].rearrange("p tt (h c) -> p tt h c", c=DH + 1)

            # ---- emission helpers ----
            def kproj(c):
                sl = slice(c * CH, (c + 1) * CH)
                xk = xpool.tile([P, KO, CH], bf16, tag="x", name=f"xk{c}")
                nc.sync.dma_start(out=xk, in_=kT_r[:, :, sl])
                for m in range(2):
                    ps = shared_ps.tile([P, CH], f32, tag="ps")
                    for ko in range(KO):
                        nc.tensor.matmul(ps, wk_sb[:, ko, ts(m, P)], xk[:, ko, :],
                                         start=(ko == 0), stop=(ko == KO - 1))
                    nc.vector.tensor_scalar_add(out=k_sb[:, m, sl], in0=ps,
                                                scalar1=bk_sb[:, m : m + 1])

            def qproj(c):
                sl = slice(c * CH, (c + 1) * CH)
                xq = xpool.tile([P, KO, CH], bf16, tag="x", name=f"xq{c}")
                nc.sync.dma_start(out=xq, in_=qT_r[:, :, sl])
                for m in range(2):
                    ps = shared_ps.tile([P, CH], f32, tag="ps")
                    for ko in range(KO):
                        nc.tensor.matmul(ps, wq_sb[:, ko, ts(m, P)], xq[:, ko, :],
                                         start=(ko == 0), stop=(ko == KO - 1))
                    nc.vector.tensor_scalar_add(out=q_sb[:, m, sl], in0=ps,
                                                scalar1=bq_sb[:, m : m + 1])

            def vproj(c):
                sl = slice(c * CH, (c + 1) * CH)
                xv = xpool.tile([P, KO, CH], bf16, tag="x", name=f"xv{c}")
                nc.sync.dma_start(out=xv, in_=vT_r[:, :, sl])
                for th in range(CH // P):
                    tt = (c * CH) // P + th
                    ps = shared_ps.tile([P, DHC], f32, tag="ps")
                    for ko in range(KO):
                        nc.tensor.matmul(ps, xv[:, ko, ts(th, P)], wv_sb[:, ko, :],
                                         start=(ko == 0), stop=(ko == KO - 1))
                    nc.vector.tensor_add(
                        out=v_heads[:, tt, :, 0:DH],
                        in0=ps.rearrange("p (h c) -> p h c", c=DH),
                        in1=bv_heads,
                    )

            # pts[c][h] SBUF tiles of exp'd logits (keys on partitions)
            pts = [dict() for _ in range(NCH)]

            def alloc_pts(c, heads):
                for h in heads:
                    pts[c][h] = ptpool.tile([P, TT, CH], bf16, tag="pt",
                                            name=f"pt_c{c}_h{h}")

            def qk_tbs(c, m, tbs):
                """QK + exp for heads (2m, 2m+1) of chunk c over tb groups.
                The two heads sit at PE row-tiles 0/64, interleaved so the HW
                runs them concurrently."""
                csl = slice(c * CH, (c + 1) * CH)
                h0, h1 = 2 * m, 2 * m + 1
                for tb in tbs:
                    psA = l_ps.tile([P, 2, CH], f32, tag="l")
                    psB = l_ps.tile([P, 2, CH], f32, tag="l")
                    for j in range(2):
                        tt = 2 * tb + j
                        nc.tensor.matmul(
                            psA[:, j, :],
                            k_sb[0:DH, m, ts(tt, P)],
                            q_sb[0:DH, m, csl],
                            start=True, stop=True,
                        )
                        nc.tensor.matmul(
                            psB[:, j, :],
                            k_sb[DH : 2 * DH, m, ts(tt, P)],
                            q_sb[DH : 2 * DH, m, csl],
                            start=True, stop=True,
                        )
                    for ps_t, hh in ((psA, h0), (psB, h1)):
                        dst = pts[c][hh][:, 2 * tb : 2 * tb + 2, :]
                        if tb in SEXP_OFF:
                            nc.vector.tensor_scalar(
                                out=dst.bitcast(i16), in0=ps_t,
                                scalar1=SEXP_A, scalar2=SEXP_B,
                                op0=MULT, op1=ADD)
                        else:
                            nc.scalar.activation(out=dst, in_=ps_t, func=EXP)

            def pv_head(c, h):
                """PV (+ denominator via ones column) and normalize for head h."""
                csl = slice(c * CH, (c + 1) * CH)
                base = DH * (h % 2)
                m = h // 2
                po = shared_ps.tile([P, CH], f32, tag="ps")
                for tt in range(TT):
                    nc.tensor.matmul(
                        po[0 : DH + 1, :],
                        v_heads[:, tt, h, :],
                        pts[c][h][:, tt, :],
                        start=(tt == 0), stop=(tt == TT - 1),
                    )
                # stage the denominator row at partition 0: the custom-DVE
                # reciprocal misreads inputs at base_partition != 0 (and PSUM
                # sources) - both hardware-verified
                den = spool.tile([1, CH], f32, tag="den")
                nc.vector.tensor_copy(out=den[0:1, :], in_=po[DH : DH + 1, :])
                rec = spool.tile([1, CH], f32, tag="rec")
                nc.vector.reciprocal_approx_fast(out=rec[0:1, :], in_=den[0:1, :])
                bc = spool.tile([P, CH], f32, tag="bc")
                nc.gpsimd.partition_broadcast(bc[0:DH, :], rec[0:1, :])
                nc.vector.tensor_mul(
                    out=attn_sb[base : base + DH, m, csl],
                    in0=po[0:DH, :], in1=bc[0:DH, :],
                )

            def outproj(c, last=False):
                tail_evac = c >= NCH - 2
                # Out-proj runs entirely through the l_ps (QK logits) ring:
                # its WAR dependencies defer these matmuls to the next chunk
                # boundary, where the evacuations no longer gate anything in
                # the ACT/DVE queues (shared_ps evacs stalled the exp stream
                # ~15us at every boundary). Evacs are DVE-only mid-stream so
                # ACT stays a pure exp queue; the idle ACT helps in the tail.
                for sti in range(CH // P):
                    st = c * (CH // P) + sti
                    pw2 = l_ps.tile([P, 2, CH], f32, tag="l")
                    for n in range(2):
                        for ko in range(2):
                            nc.tensor.matmul(pw2[:, n, :],
                                             attn_sb[:, ko, ts(st, P)],
                                             wo_sb[:, ko, ts(n, 512)],
                                             start=(ko == 0), stop=(ko == 1))
                    for n in range(2):
                        ot = opool.tile([P, 512], f32, tag="ot")
                        if tail_evac and n == 1:
                            # chunks 2/3 evacuate in the tail where ACT is idle
                            nc.scalar.copy(out=ot, in_=pw2[:, n, :])
                        else:
                            nc.vector.tensor_copy(out=ot, in_=pw2[:, n, :])
                        # sync queue is idle once inputs land; gpsimd triggers
                        # (644ns each) were delaying the normalize broadcasts
                        nc.sync.dma_start(out=out.ap()[ts(st, P), ts(n, 512)], in_=ot)

            # ---- emission schedule ----
            # Startup: K-projection chunks interleaved with chunk 0's QK
            # groups so exp starts as soon as the first key chunk is ready.
            kproj(0)
            nc.sync.dma_start(out=wq_sb, in_=wq.ap().rearrange("(ko p) m -> p ko m", p=P))
            nc.sync.dma_start(out=bq_sb, in_=bq.ap().rearrange("(mo p) -> p mo", p=P))
            qproj(0)
            alloc_pts(0, range(HC))
            qk_tbs(0, 0, [0, 1]); qk_tbs(0, 1, [0, 1])
            kproj(1)
            qk_tbs(0, 0, [2, 3]); qk_tbs(0, 1, [2, 3])
            kproj(2)
            qk_tbs(0, 0, [4, 5]); qk_tbs(0, 1, [4, 5])
            kproj(3)
            qk_tbs(0, 0, [6, 7]); qk_tbs(0, 1, [6, 7])

            ones_f32 = wpool.tile([P, TT, HC], f32, tag="ones")
            nc.vector.memset(ones_f32, 1.0)
            nc.vector.tensor_copy(out=v_heads[:, :, :, DH], in_=ones_f32)
            nc.vector.memset(v_sb[:, :, VW:], 0.0)

            # Steady state: QK of chunk c+1 ahead of PV of chunk c in the
            # in-order PE queue; PV/normalize/out of chunk c execute while
            # ACT streams chunk c+1's exps. V projection rides inside
            # iteration 0, after xq1's DMA is already queued.
            for c in range(NCH):
                if c + 1 < NCH:
                    qproj(c + 1)
                    alloc_pts(c + 1, range(HC))
                    qk_tbs(c + 1, 0, range(TT // 2))
                if c == 0:
                    nc.sync.dma_start(out=wv_sb, in_=wv.ap().rearrange("(ko p) m -> p ko m", p=P))
                    nc.sync.dma_start(out=bv_row[0:1, :], in_=bv.ap().rearrange("(a d) -> a d", a=1))
                    nc.gpsimd.partition_broadcast(bv_bc, bv_row[0:1, :])
                    for vc in range(NCH):
                        vproj(vc)
                    nc.sync.dma_start(out=wo_sb, in_=wo.ap().rearrange("(ko p) n -> p ko n", p=P))
                if not (c == NCH - 1):
                    pv_head(c, 0)
                    pv_head(c, 1)
                if c + 1 < NCH:
                    qk_tbs(c + 1, 1, range(TT // 2))
                pv_head(c, 2)
                pv_head(c, 3)
                if c + 1 == NCH - 1:
                    # last chunk's first head-pair PV runs during chunk 3's
                    # exp stream instead of serializing into the tail
                    pv_head(c + 1, 0)
                    pv_head(c + 1, 1)
                if c != NCH - 2:
                    outproj(c, last=(c == NCH - 1))
                if c == NCH - 1:
                    # chunk 2's out-proj is emitted AFTER chunk 3's: the l_ps
                    # ring otherwise makes chunk 3's (critical-chain) matmuls
                    # wait on chunk 2's evacuations; this way chunk 3's chain
                    # runs first and chunk 2 fills the trailing slack
                    outproj(NCH - 2)

    nc.finalize()
    return nc


def kernel(**inputs):
    global _compiled, last_results
    if _compiled is None:
        _compiled = _build()
    nc = _compiled

    query = np.asarray(inputs["query"], np.float32)
    key = np.asarray(inputs["key"], np.float32)
    value = np.asarray(inputs["value"], np.float32)
    Wq = np.asarray(inputs["Wq"], np.float32)
    Wk = np.asarray(inputs["Wk"], np.float32)
    Wv = np.asarray(inputs["Wv"], np.float32)
    Wo = np.asarray(inputs["Wo"], np.float32)
    bq_f = np.asarray(inputs["bq"], np.float32)
    bk_f = np.asarray(inputs["bk"], np.float32)
    bv_f = np.asarray(inputs["bv"], np.float32)
    bo_f = np.asarray(inputs["bo"], np.float32)

    bf = ml_dtypes.bfloat16
    scale = 1.0 / np.sqrt(np.float32(DH))
    qT = [np.ascontiguousarray(query[b].T).astype(bf) for b in range(B)]
    kT = [np.ascontiguousarray(key[b].T).astype(bf) for b in range(B)]
    vT = [np.ascontiguousarray(value[b].T).astype(bf) for b in range(B)]

    in_maps = []
    for c in range(NCORES):
        b = c // 4
        sh = c % 4
        sl = slice(DHC * sh, DHC * (sh + 1))
        in_maps.append({
            "qT": qT[b], "kT": kT[b], "vT": vT[b],
            "wq": (Wq[:, sl] * scale).astype(bf),
            "wk": np.ascontiguousarray(Wk[:, sl]).astype(bf),
            "wv": np.ascontiguousarray(Wv[:, sl]).astype(bf),
            "wo": np.ascontiguousarray(Wo[sl, :]).astype(bf),
            "bq": np.ascontiguousarray(bq_f[sl]) * scale,
            "bk": np.ascontiguousarray(bk_f[sl]),
            "bv": np.ascontiguousarray(bv_f[sl]),
        })

    res = bass_utils.run_bass_kernel_spmd(nc, in_maps, core_ids=list(range(NCORES)))
    last_results = res

    final = np.empty((B, S, D), np.float32)
    for b in range(B):
        acc = res.results[4 * b]["out"].astype(np.float32)
        for sh in range(1, 4):
            acc = acc + res.results[4 * b + sh]["out"]
        final[b] = acc + bo_f
    return final
